# revision 20
# baseline (speedup 1.0000x reference)
"""Multi-head causal attention (B=8, S=1024, E=512, H=8, Dk=Dv=64) on 8 NeuronCores.

Sharding: data-parallel over batch. Core b computes the full attention block
for X[b]; no collectives. Host pre-transposes X[b] -> [E, S], converts matmul
operands to bf16, and pre-arranges weights so the device kernel is pure
matmul + softmax.

v3 structure:
  - PE warmup matmuls at t=0 flip the HAM clock-gate to 8/8 during input DMA;
    input DMAs are ordered by first use and spread across sync/scalar/gpsimd
    descriptor queues.
  - The attention inner loop is ACT(exp)-bound, so projection work is
    interleaved INTO it as "filler units" (4 matmuls + one PSUM->SBUF convert
    each, all [128,512]) emitted between a k-block's scores and the previous
    block's AV matmuls.  This keeps TensorE dense (no HAM re-throttle) and
    software-pipelines the exp latency.
  - bv is folded into the output bias on the host (A@(V + 1 bv^T)/d = A@V/d +
    bv exactly, since the ones-column denominator divides out), so the V
    convert is a plain copy.
  - Causal diagonal: the exp'd triangle is zeroed by one DVE multiply with a
    0/1 slab per diagonal block (no PE mask matmuls).
  - PSUM: tag "st" 2x[128,1024] (scores double-buffer + upfront projection
    pairs), tag "ot" 3x[65,512] (AV accumulators overlap the normalize
    chain across head-pairs), tag "yw" 1x[128,512] (warmup + filler units).

Per-core dataflow (bf16 matmuls, fp32 PSUM accumulate / softmax math):
  V = X @ Wv with a ones column per head (AV emits softmax denominators)
  QT/KT = (W^T X)^T per head-pair -> [128 dd, 1024 q] (+bq/bk per partition)
  per head-pair, q-chunk, k-block: scores^T via two row-tiled K=64 matmuls,
    exp on ScalarE (scale=1/8), DVE triangle zero on diagonal blocks,
    O^T accum = V^T @ exp (65th row = denominator), O^T *= 1/denom via
    fast-NR reciprocal + gpsimd partition broadcast.
  Y[s-chunk] = sum_p O^T-chunk^T @ Wo_p + (bo + bv@Wo), one DMA per 128 rows.
"""

import numpy as np
import ml_dtypes

import concourse.bass as bass
import concourse.tile as tile
import concourse.mybir as mybir
from concourse import bacc
from concourse import bass_utils

B, S, E = 8, 1024, 512
H, DK, DV = 8, 64, 64
HD = H * DK  # 512
P = 128
EC = E // P  # 4 contraction chunks over E
NPAIR = H // 2
NCORES = 8
F32 = mybir.dt.float32
BF16 = mybir.dt.bfloat16

_COMPILED = None


def _body(nc, tc, const, work, ps, d):
    # ---- PE warmup: TensorE busy from t=0 so HAM goes 8/8 during input DMA ----
    wu_src = const.tile([P, 512], BF16, tag="wusrc", name="wu_src")
    nc.vector.memset(wu_src[:], 0.125)
    for i in range(9):
        wu = ps.tile([P, 512], F32, tag="yw", bufs=1, name=f"wu{i}")
        nc.tensor.matmul(wu[:], wu_src[:, 0:P], wu_src[:],
                         start=True, stop=True, skip_group_check=True)

    # ---- SBUF tiles for inputs (pair-consolidated for big DMAs) ----
    xt01 = const.tile([P, 2, S], BF16, tag="xt01", name="xt01")
    xt23 = const.tile([P, 2, S], BF16, tag="xt23", name="xt23")
    wv01 = const.tile([P, 2, HD], BF16, tag="wv01", name="wv01")
    wv23 = const.tile([P, 2, HD], BF16, tag="wv23", name="wv23")
    wqa = const.tile([P, EC, HD], BF16, tag="wqa", name="wqa")
    wka = const.tile([P, EC, HD], BF16, tag="wka", name="wka")
    woa = const.tile([P, EC, E], BF16, tag="woa", name="woa")
    bq_t = const.tile([P, NPAIR], F32, tag="bq", name="bq_t")
    bk_t = const.tile([P, NPAIR], F32, tag="bk", name="bk_t")
    bob_t = const.tile([P, E], F32, tag="bob", name="bob_t")
    tri2_t = const.tile([P, 2 * P], BF16, tag="tri2", name="tri2_t")
    wo3hi_t = const.tile([DV, E], BF16, tag="wo3hi", name="wo3hi_t")

    def xts(c):
        return (xt01 if c < 2 else xt23)[:, c % 2, :]

    def wvs(c):
        return (wv01 if c < 2 else wv23)[:, c % 2, :]

    wq_sb = [wqa[:, c, :] for c in range(EC)]
    wk_sb = [wka[:, c, :] for c in range(EC)]
    wo_sb = [woa[:, c, :] for c in range(EC)]
    xt = [xts(c) for c in range(EC)]
    wv_sb = [wvs(c) for c in range(EC)]

    # ---- input DMAs: few big transfers, ordered by first use, 3 queues ----
    def dview(t, c0, n):
        return d[t][c0 * P:(c0 + n) * P, :].rearrange("(c p) s -> p c s", c=n)

    nc.sync.dma_start(wv01[:], dview("wv", 0, 2))
    nc.sync.dma_start(xt01[:], dview("xt", 0, 2))
    nc.scalar.dma_start(wv23[:], dview("wv", 2, 2))
    nc.scalar.dma_start(xt23[:], dview("xt", 2, 2))
    nc.sync.dma_start(wqa[:], dview("wq", 0, 4))
    nc.scalar.dma_start(wka[:], dview("wk", 0, 4))
    nc.sync.dma_start(bq_t[:], d["bq"][:])
    nc.scalar.dma_start(bk_t[:], d["bk"][:])
    nc.gpsimd.dma_start(tri2_t[:], d["tri2"][:])
    nc.gpsimd.dma_start(woa[:], dview("wo", 0, 4))
    nc.gpsimd.dma_start(bob_t[:], d["bob"][:])
    nc.gpsimd.dma_start(wo3hi_t[:], d["wo3hi"][:])

    # ---- persistent SBUF results ----
    # vd[j]: [128 k, 2 x (8 heads x 65)] bf16, ones column per head
    vd = [const.tile([P, 2 * 520], BF16, tag=f"vd{j}", name=f"vd{j}") for j in range(4)]
    for j in range(4):
        nc.vector.memset(vd[j][:], 1.0)
    qt = {p: const.tile([P, 1024], BF16, tag=f"qt{p}", name=f"qt{p}")
          for p in range(NPAIR)}
    kt = {p: const.tile([P, 1024], BF16, tag=f"kt{p}", name=f"kt{p}")
          for p in range(NPAIR)}
    ot_sb = {}

    def v_copy(j, sh, src):
        t3o = vd[j][:, sh * 520:(sh + 1) * 520].rearrange("p (h c) -> p h c", c=65)
        nc.vector.tensor_copy(
            t3o[:, :, 0:DV], src.rearrange("p (h c) -> p h c", c=DV))

    # ---- upfront projections (paired [128,1024] PSUM tiles, no stalls):
    # V for si 0..3 (k-blocks 0..3) and Q/K for pair 0, q-chunk 0 ----
    def v_pair(j):
        vp = ps.tile([P, 1024], F32, tag="st", name=f"vp{j}")
        for sh in range(2):
            si = 2 * j + sh
            for c in range(EC):
                nc.tensor.matmul(
                    vp[:, sh * 512:(sh + 1) * 512],
                    xt[c][:, si * P:(si + 1) * P], wv_sb[c][:],
                    start=(c == 0), stop=(c == EC - 1))
        for sh in range(2):
            v_copy(j, sh, vp[:, sh * 512:(sh + 1) * 512])

    v_pair(0)
    qkp = ps.tile([P, 1024], F32, tag="st", name="qkp0")
    for c in range(EC):
        nc.tensor.matmul(qkp[:, 0:512], wq_sb[c][:, 0:P], xt[c][:, 0:512],
                         start=(c == 0), stop=(c == EC - 1))
    for c in range(EC):
        nc.tensor.matmul(qkp[:, 512:1024], wk_sb[c][:, 0:P], xt[c][:, 0:512],
                         start=(c == 0), stop=(c == EC - 1))
    nc.scalar.add(qt[0][:, 0:512], qkp[:, 0:512], bq_t[:, 0:1])
    nc.vector.tensor_scalar_add(kt[0][:, 0:512], qkp[:, 512:1024], bk_t[:, 0:1])

    # ---- filler units: 4 matmuls + 1 convert each, PSUM tag "yw" ----
    def unit_qk(p, qc, which):
        def emit():
            t = ps.tile([P, 512], F32, tag="yw", bufs=1, name=f"u{which}{p}{qc}")
            w = wq_sb if which == "q" else wk_sb
            for c in range(EC):
                nc.tensor.matmul(
                    t[:], w[c][:, p * P:(p + 1) * P],
                    xt[c][:, qc * 512:(qc + 1) * 512],
                    start=(c == 0), stop=(c == EC - 1))
            if which == "q":
                nc.scalar.add(qt[p][:, qc * 512:(qc + 1) * 512], t[:], bq_t[:, p:p + 1])
            else:
                nc.vector.tensor_scalar_add(
                    kt[p][:, qc * 512:(qc + 1) * 512], t[:], bk_t[:, p:p + 1])
        return emit

    def unit_v(j, sh):
        def emit():
            si = 2 * j + sh
            t = ps.tile([P, 512], F32, tag="yw", bufs=1, name=f"uv{si}")
            for c in range(EC):
                nc.tensor.matmul(t[:], xt[c][:, si * P:(si + 1) * P], wv_sb[c][:],
                                 start=(c == 0), stop=(c == EC - 1))
            v_copy(j, sh, t[:])
        return emit

    def unit_yp(qc, sj):
        def emit():
            si = qc * 4 + sj
            sl = si % 4
            t = ps.tile([P, 512], F32, tag="yw", bufs=1, name=f"uy{si}")
            for p in range(NPAIR):
                nc.tensor.matmul(
                    t[:], ot_sb[p, qc][:, sl * P:(sl + 1) * P], wo_sb[p][:],
                    start=(p == 0), stop=(p == NPAIR - 1))
            yo = work.tile([P, E], F32, tag="yo", name=f"yo{si}", bufs=4)
            nc.vector.tensor_add(yo[:], t[:], bob_t[:])
            nc.sync.dma_start(d["y"][si * P:(si + 1) * P, :], yo[:])
        return emit

    tmp31 = {}

    # ---- attention for one (head-pair, q-chunk); fills[ki] emitted between
    # the k-block's scores and the PREVIOUS block's AV (latency hiding) ----
    def attn(p, qc, fills):
        n_ki = 4 * (qc + 1)
        otp = {}
        for hb in range(2):
            otp[hb] = ps.tile([DV + 1, 512], F32, tag="ot", bufs=3,
                              name=f"otp{p}_{qc}_{hb}")
        stes = {}

        def emit_av(ki):
            off = max(ki * P - qc * 512, 0)
            st_f, sp_f = (ki == 0), (ki == n_ki - 1)
            for hb in range(2):
                h = 2 * p + hb
                vsl = (ki % 2) * 520 + h * 65
                nc.tensor.matmul(
                    otp[hb][:, off:], vd[ki // 2][:, vsl:vsl + 65],
                    stes[ki][:, hb * 512 + off:(hb + 1) * 512],
                    start=st_f, stop=sp_f, skip_group_check=True)

        pend = None
        for ki in range(n_ki):
            diag = (ki * P - qc * 512) >= 0
            off = max(ki * P - qc * 512, 0)
            stp = ps.tile([P, 1024], F32, tag="st", name=f"st{p}_{qc}_{ki}")
            for hb in range(2):
                hp = slice(hb * DK, (hb + 1) * DK)
                nc.tensor.matmul(
                    stp[:, hb * 512 + off:(hb + 1) * 512],
                    kt[p][hp, ki * P:(ki + 1) * P],
                    qt[p][hp, qc * 512 + off:(qc + 1) * 512],
                    start=True, stop=True, tile_position=(hb * DK, 0),
                    skip_group_check=True)
            if pend is not None:
                emit_av(pend)
            for u in fills.get(ki, []):
                u()
            ste = work.tile([P, 1024], BF16, tag="ste", name=f"ste{p}_{qc}_{ki}", bufs=6)
            stes[ki] = ste
            stp3 = stp.rearrange("p (h q) -> p h q", h=2)[:, :, off:]
            ste3 = ste.rearrange("p (h q) -> p h q", h=2)[:, :, off:]
            nc.scalar.activation(
                ste3, stp3, mybir.ActivationFunctionType.Exp, scale=0.125)
            if diag:
                nc.vector.tensor_mul(
                    ste3[:, :, 0:P], ste3[:, :, 0:P],
                    tri2_t.rearrange("p (h q) -> p h q", h=2))
            pend = ki
        emit_av(pend)

        # normalize: O^T *= 1/denominator (row DV of each accumulator)
        ot = const.tile([P, 512], BF16, tag=f"ot{p}_{qc}", name=f"ot{p}_{qc}")
        for hb in (1, 0):
            h = 2 * p + hb
            rrow = work.tile([1, 512], F32, tag="rrow", name=f"rrow{h}_{qc}", bufs=4)
            nc.vector.tensor_copy(rrow[:], otp[hb][DV:DV + 1, :])
            rec = work.tile([1, 512], F32, tag="rec", name=f"rec{h}_{qc}", bufs=4)
            nc.vector.reciprocal_approx_fast(rec[:], rrow[:])
            rb = work.tile([DV, 512], F32, tag="rb", name=f"rb{h}_{qc}", bufs=4)
            nc.gpsimd.partition_broadcast(rb[:], rec[:])
            if hb == 0:
                nc.vector.tensor_mul(ot[0:DV, :], otp[0][0:DV, :], rb[:])
            else:
                # DVE cannot shift partitions: scale into a temp at base 0,
                # then SBUF->SBUF DMA into partitions 64-127 of the pair tile.
                # For the last pair the consumer reads the temp directly.
                tmp = work.tile([DV, 512], BF16, tag="ottmp",
                                name=f"ottmp{p}_{qc}", bufs=4)
                nc.vector.tensor_mul(tmp[:], otp[1][0:DV, :], rb[:])
                if (p, qc) == (3, 1):
                    tmp31[0] = tmp
                else:
                    nc.scalar.dma_start(ot[DV:P, :], tmp[:])
        ot_sb[p, qc] = ot

    # ---- schedule: attention with projection/output fillers threaded in ----
    attn(0, 0, {0: [unit_v(1, 0)], 1: [unit_v(1, 1)],
                2: [unit_qk(1, 0, "q")], 3: [unit_qk(1, 0, "k")]})
    attn(1, 0, {0: [unit_qk(2, 0, "q")], 2: [unit_qk(2, 0, "k")]})
    attn(2, 0, {0: [unit_qk(3, 0, "q")], 2: [unit_qk(3, 0, "k")]})
    attn(3, 0, {0: [unit_qk(0, 1, "q")], 2: [unit_qk(0, 1, "k")]})
    attn(0, 1, {0: [unit_v(2, 0)], 1: [unit_v(2, 1)], 2: [unit_v(3, 0)],
                3: [unit_v(3, 1)], 5: [unit_qk(1, 1, "q")],
                7: [unit_qk(1, 1, "k")]})
    attn(1, 1, {2: [unit_qk(2, 1, "q")], 5: [unit_qk(2, 1, "k")]})
    attn(2, 1, {1: [unit_qk(3, 1, "q")], 3: [unit_qk(3, 1, "k")],
                5: [unit_yp(0, 0)], 7: [unit_yp(0, 1)]})
    attn(3, 1, {2: [unit_yp(0, 2)], 5: [unit_yp(0, 3)]})
    # final output projection (si 4..7): two paired PSUM tiles; pairs 0-2
    # accumulate while pair (3,1)'s normalize chain runs, pair 3 lands last
    yps = []
    for g in range(2):
        yp = ps.tile([P, 1024], F32, tag="st", name=f"ypf{g}")
        for sh in range(2):
            sl = g * 2 + sh
            for p in range(3):
                nc.tensor.matmul(
                    yp[:, sh * 512:(sh + 1) * 512],
                    ot_sb[p, 1][:, sl * P:(sl + 1) * P], wo_sb[p][:],
                    start=(p == 0), stop=False, skip_group_check=True)
        yps.append(yp)
    # keep TensorE warm (HAM 8/8) while the last normalize chain runs
    for i in range(10):
        wu = ps.tile([P, 512], F32, tag="yw", bufs=1, name=f"kw{i}")
        nc.tensor.matmul(wu[:], wu_src[:, 0:P], wu_src[:],
                         start=True, stop=True, skip_group_check=True)
    engs = [nc.sync, nc.scalar, nc.sync, nc.scalar]
    for g in range(2):
        for sh in range(2):
            sl = g * 2 + sh
            si = 4 + sl
            nc.tensor.matmul(
                yps[g][:, sh * 512:(sh + 1) * 512],
                ot_sb[3, 1][0:DV, sl * P:(sl + 1) * P], wo_sb[3][0:DV, :],
                start=False, stop=False, skip_group_check=True)
            nc.tensor.matmul(
                yps[g][:, sh * 512:(sh + 1) * 512],
                tmp31[0][:, sl * P:(sl + 1) * P], wo3hi_t[:],
                start=False, stop=True, skip_group_check=True)
            yo = work.tile([P, E], F32, tag="yof", name=f"yof{si}", bufs=4)
            nc.vector.tensor_add(
                yo[:], yps[g][:, sh * 512:(sh + 1) * 512], bob_t[:])
            engs[sl].dma_start(d["y"][si * P:(si + 1) * P, :], yo[:])


def _build():
    nc = bacc.Bacc("TRN2", target_bir_lowering=False, debug=False)
    d = {
        "xt": nc.dram_tensor("xt", [E, S], BF16, kind="ExternalInput").ap(),
        "wq": nc.dram_tensor("wq", [E, HD], BF16, kind="ExternalInput").ap(),
        "wk": nc.dram_tensor("wk", [E, HD], BF16, kind="ExternalInput").ap(),
        "wv": nc.dram_tensor("wv", [E, HD], BF16, kind="ExternalInput").ap(),
        "wo": nc.dram_tensor("wo", [HD, E], BF16, kind="ExternalInput").ap(),
        "tri2": nc.dram_tensor("tri2", [P, 2 * P], BF16, kind="ExternalInput").ap(),
        "bq": nc.dram_tensor("bq", [P, NPAIR], F32, kind="ExternalInput").ap(),
        "bk": nc.dram_tensor("bk", [P, NPAIR], F32, kind="ExternalInput").ap(),
        "bob": nc.dram_tensor("bob", [P, E], F32, kind="ExternalInput").ap(),
        "wo3hi": nc.dram_tensor("wo3hi", [DV, E], BF16, kind="ExternalInput").ap(),
        "y": nc.dram_tensor("y", [S, E], F32, kind="ExternalOutput").ap(),
    }
    with tile.TileContext(nc) as tc:
        with tc.tile_pool(name="const", bufs=1) as const, \
             tc.tile_pool(name="work", bufs=3) as work, \
             tc.tile_pool(name="ps", bufs=2, space="PSUM") as ps:
            _body(nc, tc, const, work, ps, d)
    nc.compile()
    return nc


def get_nc():
    global _COMPILED
    if _COMPILED is None:
        _COMPILED = _build()
    return _COMPILED


def _prep_in_maps(X, Wq, bq, Wk, bk, Wv, bv, Wo, bo):
    f = np.float32
    bf = ml_dtypes.bfloat16
    Wof = np.asarray(Wo, f).reshape(HD, E)
    # A@(V + 1 bv^T)/d = A@V/d + bv exactly (the ones-column denominator
    # divides out), so bv contributes bv_concat @ Wo to every output row.
    bo_eff = np.asarray(bo, f).reshape(E) + np.asarray(bv, f).reshape(HD) @ Wof
    shared = {
        "wq": np.ascontiguousarray(
            np.transpose(np.asarray(Wq, f), (1, 0, 2)).reshape(E, HD).astype(bf)),
        "wk": np.ascontiguousarray(
            np.transpose(np.asarray(Wk, f), (1, 0, 2)).reshape(E, HD).astype(bf)),
        "wv": np.ascontiguousarray(
            np.transpose(np.asarray(Wv, f), (1, 0, 2)).reshape(E, HD).astype(bf)),
        "wo": np.ascontiguousarray(Wof.astype(bf)),
        "bq": np.ascontiguousarray(np.asarray(bq, f).reshape(HD).reshape(NPAIR, P).T),
        "bk": np.ascontiguousarray(np.asarray(bk, f).reshape(HD).reshape(NPAIR, P).T),
        "bob": np.ascontiguousarray(np.broadcast_to(bo_eff.reshape(1, E), (P, E))),
        "wo3hi": np.ascontiguousarray(Wof[HD - DV:HD, :].astype(bf)),
    }
    # 0/1 keep-mask for the diagonal 128x128 triangle (keep k <= q), twice
    # side by side so one DVE op covers both heads
    keep = np.triu(np.ones((P, P), dtype=f))
    shared["tri2"] = np.ascontiguousarray(np.tile(keep, (1, 2)).astype(bf))
    Xf = np.asarray(X, f)
    in_maps = []
    for b in range(B):
        m = dict(shared)
        m["xt"] = np.ascontiguousarray(Xf[b].T.astype(bf))
        in_maps.append(m)
    return in_maps


def kernel(X, Wq, bq, Wk, bk, Wv, bv, Wo, bo):
    nc = get_nc()
    in_maps = _prep_in_maps(X, Wq, bq, Wk, bk, Wv, bv, Wo, bo)
    res = bass_utils.run_bass_kernel_spmd(nc, in_maps, core_ids=list(range(NCORES)))
    return np.stack([res.results[b]["y"] for b in range(B)], axis=0).astype(np.float32)


def run_traced(X, Wq, bq, Wk, bk, Wv, bv, Wo, bo):
    """Like kernel() but with NTFF profiling; returns (out, exec_time_ns)."""
    nc = get_nc()
    in_maps = _prep_in_maps(X, Wq, bq, Wk, bk, Wv, bv, Wo, bo)
    res = bass_utils.run_bass_kernel_spmd(
        nc, in_maps, core_ids=list(range(NCORES)), trace=True)
    out = np.stack([res.results[b]["y"] for b in range(B)], axis=0).astype(np.float32)
    return out, res.exec_time_ns


# revision 21
# speedup vs baseline: 1.1379x; 1.1379x over previous
"""Multi-head causal attention (B=8, S=1024, E=512, H=8, Dk=Dv=64) on 8 NeuronCores.

Sharding: data-parallel over batch. Core b computes the full attention block
for X[b]; no collectives. Host pre-transposes X[b] -> [E, S], converts matmul
operands to bf16, and pre-arranges weights so the device kernel is pure
matmul + softmax.

v3 structure:
  - PE warmup matmuls at t=0 flip the HAM clock-gate to 8/8 during input DMA;
    input DMAs are ordered by first use and spread across sync/scalar/gpsimd
    descriptor queues.
  - The attention inner loop is ACT(exp)-bound, so projection work is
    interleaved INTO it as "filler units" (4 matmuls + one PSUM->SBUF convert
    each, all [128,512]) emitted between a k-block's scores and the previous
    block's AV matmuls.  This keeps TensorE dense (no HAM re-throttle) and
    software-pipelines the exp latency.
  - bv is folded into the output bias on the host (A@(V + 1 bv^T)/d = A@V/d +
    bv exactly, since the ones-column denominator divides out), so the V
    convert is a plain copy.
  - Causal diagonal: the exp'd triangle is zeroed by one DVE multiply with a
    0/1 slab per diagonal block (no PE mask matmuls).
  - PSUM: tag "st" 2x[128,1024] (scores double-buffer + upfront projection
    pairs), tag "ot" 3x[65,512] (AV accumulators overlap the normalize
    chain across head-pairs), tag "yw" 1x[128,512] (warmup + filler units).

Per-core dataflow (bf16 matmuls, fp32 PSUM accumulate / softmax math):
  V = X @ Wv with a ones column per head (AV emits softmax denominators)
  QT/KT = (W^T X)^T per head-pair -> [128 dd, 1024 q] (+bq/bk per partition)
  per head-pair, q-chunk, k-block: scores^T via two row-tiled K=64 matmuls,
    exp on ScalarE (scale=1/8), DVE triangle zero on diagonal blocks,
    O^T accum = V^T @ exp (65th row = denominator), O^T *= 1/denom via
    fast-NR reciprocal + gpsimd partition broadcast.
  Y[s-chunk] = sum_p O^T-chunk^T @ Wo_p + (bo + bv@Wo), one DMA per 128 rows.
"""

import numpy as np
import ml_dtypes

import concourse.bass as bass
import concourse.tile as tile
import concourse.mybir as mybir
from concourse import bacc
from concourse import bass_utils

B, S, E = 8, 1024, 512
H, DK, DV = 8, 64, 64
HD = H * DK  # 512
P = 128
EC = E // P  # 4 contraction chunks over E
NPAIR = H // 2
NCORES = 8
F32 = mybir.dt.float32
BF16 = mybir.dt.bfloat16

_COMPILED = None


def _body(nc, tc, const, work, ps, d):
    # ---- PE warmup: TensorE busy from t=0 so HAM goes 8/8 during input DMA ----
    wu_src = const.tile([P, 512], BF16, tag="wusrc", name="wu_src")
    nc.vector.memset(wu_src[:], 0.125)
    for i in range(9):
        wu = ps.tile([P, 512], F32, tag="yw", bufs=1, name=f"wu{i}")
        nc.tensor.matmul(wu[:], wu_src[:, 0:P], wu_src[:],
                         start=True, stop=True, skip_group_check=True)

    # ---- SBUF tiles for inputs (pair-consolidated for big DMAs) ----
    xt01 = const.tile([P, 2, S], BF16, tag="xt01", name="xt01")
    xt23 = const.tile([P, 2, S], BF16, tag="xt23", name="xt23")
    wv01 = const.tile([P, 2, HD], BF16, tag="wv01", name="wv01")
    wv23 = const.tile([P, 2, HD], BF16, tag="wv23", name="wv23")
    wqa = const.tile([P, EC, HD], BF16, tag="wqa", name="wqa")
    wka = const.tile([P, EC, HD], BF16, tag="wka", name="wka")
    woa = const.tile([P, EC, E], BF16, tag="woa", name="woa")
    bq_t = const.tile([P, NPAIR], F32, tag="bq", name="bq_t")
    bk_t = const.tile([P, NPAIR], F32, tag="bk", name="bk_t")
    bob_t = const.tile([P, E], F32, tag="bob", name="bob_t")
    tri2_t = const.tile([P, 2 * P], BF16, tag="tri2", name="tri2_t")
    wo3hi_t = const.tile([DV, E], BF16, tag="wo3hi", name="wo3hi_t")

    def xts(c):
        return (xt01 if c < 2 else xt23)[:, c % 2, :]

    def wvs(c):
        return (wv01 if c < 2 else wv23)[:, c % 2, :]

    wq_sb = [wqa[:, c, :] for c in range(EC)]
    wk_sb = [wka[:, c, :] for c in range(EC)]
    wo_sb = [woa[:, c, :] for c in range(EC)]
    xt = [xts(c) for c in range(EC)]
    wv_sb = [wvs(c) for c in range(EC)]

    # ---- input DMAs: few big transfers, ordered by first use, 3 queues ----
    def dview(t, c0, n):
        return d[t][c0 * P:(c0 + n) * P, :].rearrange("(c p) s -> p c s", c=n)

    nc.sync.dma_start(wv01[:], dview("wv", 0, 2))
    nc.sync.dma_start(xt01[:], dview("xt", 0, 2))
    nc.scalar.dma_start(wv23[:], dview("wv", 2, 2))
    nc.scalar.dma_start(xt23[:], dview("xt", 2, 2))
    nc.sync.dma_start(wqa[:], dview("wq", 0, 4))
    nc.scalar.dma_start(wka[:], dview("wk", 0, 4))
    nc.sync.dma_start(bq_t[:], d["bq"][:])
    nc.scalar.dma_start(bk_t[:], d["bk"][:])
    nc.gpsimd.dma_start(tri2_t[:], d["tri2"][:])
    nc.gpsimd.dma_start(woa[:], dview("wo", 0, 4))
    nc.gpsimd.dma_start(bob_t[:], d["bob"][:])
    nc.gpsimd.dma_start(wo3hi_t[:], d["wo3hi"][:])

    # ---- persistent SBUF results ----
    # vd[j]: [128 k, 2 x (8 heads x 65)] bf16, ones column per head
    vd = [const.tile([P, 2 * 520], BF16, tag=f"vd{j}", name=f"vd{j}") for j in range(4)]
    for j in range(4):
        nc.vector.memset(vd[j][:], 1.0)
    qt = {p: const.tile([P, 1024], BF16, tag=f"qt{p}", name=f"qt{p}")
          for p in range(NPAIR)}
    kt = {p: const.tile([P, 1024], BF16, tag=f"kt{p}", name=f"kt{p}")
          for p in range(NPAIR)}
    ot_sb = {}

    def v_copy(j, sh, src):
        t3o = vd[j][:, sh * 520:(sh + 1) * 520].rearrange("p (h c) -> p h c", c=65)
        nc.vector.tensor_copy(
            t3o[:, :, 0:DV], src.rearrange("p (h c) -> p h c", c=DV))

    # ---- upfront projections (paired [128,1024] PSUM tiles, no stalls):
    # V for si 0..3 (k-blocks 0..3) and Q/K for pair 0, q-chunk 0 ----
    def v_pair(j):
        vp = ps.tile([P, 1024], F32, tag="st", name=f"vp{j}")
        for sh in range(2):
            si = 2 * j + sh
            for c in range(EC):
                nc.tensor.matmul(
                    vp[:, sh * 512:(sh + 1) * 512],
                    xt[c][:, si * P:(si + 1) * P], wv_sb[c][:],
                    start=(c == 0), stop=(c == EC - 1))
        for sh in range(2):
            v_copy(j, sh, vp[:, sh * 512:(sh + 1) * 512])

    v_pair(0)
    qkp = ps.tile([P, 1024], F32, tag="st", name="qkp0")
    for c in range(EC):
        nc.tensor.matmul(qkp[:, 0:512], wq_sb[c][:, 0:P], xt[c][:, 0:512],
                         start=(c == 0), stop=(c == EC - 1))
    for c in range(EC):
        nc.tensor.matmul(qkp[:, 512:1024], wk_sb[c][:, 0:P], xt[c][:, 0:512],
                         start=(c == 0), stop=(c == EC - 1))
    nc.scalar.add(qt[0][:, 0:512], qkp[:, 0:512], bq_t[:, 0:1])
    nc.vector.tensor_scalar_add(kt[0][:, 0:512], qkp[:, 512:1024], bk_t[:, 0:1])

    # ---- filler units: 4 matmuls + 1 convert each, PSUM tag "yw" ----
    def unit_qk(p, qc, which):
        def emit():
            t = ps.tile([P, 512], F32, tag="yw", bufs=1, name=f"u{which}{p}{qc}")
            w = wq_sb if which == "q" else wk_sb
            for c in range(EC):
                nc.tensor.matmul(
                    t[:], w[c][:, p * P:(p + 1) * P],
                    xt[c][:, qc * 512:(qc + 1) * 512],
                    start=(c == 0), stop=(c == EC - 1))
            if which == "q":
                nc.scalar.add(qt[p][:, qc * 512:(qc + 1) * 512], t[:], bq_t[:, p:p + 1])
            else:
                nc.vector.tensor_scalar_add(
                    kt[p][:, qc * 512:(qc + 1) * 512], t[:], bk_t[:, p:p + 1])
        return emit

    def unit_v(j, sh):
        def emit():
            si = 2 * j + sh
            t = ps.tile([P, 512], F32, tag="yw", bufs=1, name=f"uv{si}")
            for c in range(EC):
                nc.tensor.matmul(t[:], xt[c][:, si * P:(si + 1) * P], wv_sb[c][:],
                                 start=(c == 0), stop=(c == EC - 1))
            v_copy(j, sh, t[:])
        return emit

    def unit_yp(qc, sj):
        def emit():
            si = qc * 4 + sj
            sl = si % 4
            t = ps.tile([P, 512], F32, tag="yw", bufs=1, name=f"uy{si}")
            for p in range(NPAIR):
                nc.tensor.matmul(
                    t[:], ot_sb[p, qc][:, sl * P:(sl + 1) * P], wo_sb[p][:],
                    start=(p == 0), stop=(p == NPAIR - 1))
            yo = work.tile([P, E], F32, tag="yo", name=f"yo{si}", bufs=2)
            nc.vector.tensor_add(yo[:], t[:], bob_t[:])
            nc.sync.dma_start(d["y"][si * P:(si + 1) * P, :], yo[:])
        return emit

    tmp31 = {}

    # ---- attention for one (head-pair, q-chunk); fills[ki] emitted between
    # the k-block's scores and the PREVIOUS block's AV (latency hiding) ----
    def attn(p, qc, fills):
        n_ki = 4 * (qc + 1)
        otp = {}
        for hb in range(2):
            otp[hb] = ps.tile([DV + 1, 512], F32, tag="ot", bufs=3,
                              name=f"otp{p}_{qc}_{hb}")
        stes = {}

        def emit_av(ki):
            off = max(ki * P - qc * 512, 0)
            st_f, sp_f = (ki == 0), (ki == n_ki - 1)
            for hb in range(2):
                h = 2 * p + hb
                vsl = (ki % 2) * 520 + h * 65
                nc.tensor.matmul(
                    otp[hb][:, off:], vd[ki // 2][:, vsl:vsl + 65],
                    stes[ki][:, hb * 512 + off:(hb + 1) * 512],
                    start=st_f, stop=sp_f, skip_group_check=True)

        pend = None
        for ki in range(n_ki):
            diag = (ki * P - qc * 512) >= 0
            off = max(ki * P - qc * 512, 0)
            stp = ps.tile([P, 1024], F32, tag="st", name=f"st{p}_{qc}_{ki}")
            for hb in range(2):
                hp = slice(hb * DK, (hb + 1) * DK)
                nc.tensor.matmul(
                    stp[:, hb * 512 + off:(hb + 1) * 512],
                    kt[p][hp, ki * P:(ki + 1) * P],
                    qt[p][hp, qc * 512 + off:(qc + 1) * 512],
                    start=True, stop=True, tile_position=(hb * DK, 0),
                    skip_group_check=True)
            if pend is not None:
                emit_av(pend)
            for u in fills.get(ki, []):
                u()
            ste = work.tile([P, 1024], BF16, tag="ste", name=f"ste{p}_{qc}_{ki}", bufs=6)
            stes[ki] = ste
            stp3 = stp.rearrange("p (h q) -> p h q", h=2)[:, :, off:]
            ste3 = ste.rearrange("p (h q) -> p h q", h=2)[:, :, off:]
            nc.scalar.activation(
                ste3, stp3, mybir.ActivationFunctionType.Exp, scale=0.125)
            if diag:
                nc.vector.tensor_mul(
                    ste3[:, :, 0:P], ste3[:, :, 0:P],
                    tri2_t.rearrange("p (h q) -> p h q", h=2))
            pend = ki
        emit_av(pend)

        # normalize: O^T *= 1/denominator (row DV of each accumulator)
        ot = const.tile([P, 512], BF16, tag=f"ot{p}_{qc}", name=f"ot{p}_{qc}")
        for hb in (1, 0):
            h = 2 * p + hb
            rrow = work.tile([1, 512], F32, tag="rrow", name=f"rrow{h}_{qc}", bufs=4)
            nc.vector.tensor_copy(rrow[:], otp[hb][DV:DV + 1, :])
            rec = work.tile([1, 512], F32, tag="rec", name=f"rec{h}_{qc}", bufs=4)
            nc.vector.reciprocal_approx_fast(rec[:], rrow[:])
            rb = work.tile([DV, 512], F32, tag="rb", name=f"rb{h}_{qc}", bufs=4)
            nc.gpsimd.partition_broadcast(rb[:], rec[:])
            if hb == 0:
                nc.vector.tensor_mul(ot[0:DV, :], otp[0][0:DV, :], rb[:])
            else:
                # DVE cannot shift partitions: scale into a temp at base 0,
                # then SBUF->SBUF DMA into partitions 64-127 of the pair tile.
                # For the last pair the consumer reads the temp directly.
                tmp = work.tile([DV, 512], BF16, tag="ottmp",
                                name=f"ottmp{p}_{qc}", bufs=4)
                nc.vector.tensor_mul(tmp[:], otp[1][0:DV, :], rb[:])
                if (p, qc) == (3, 1):
                    tmp31[0] = tmp
                else:
                    nc.scalar.dma_start(ot[DV:P, :], tmp[:])
        ot_sb[p, qc] = ot

    # ---- schedule: attention with projection/output fillers threaded in ----
    attn(0, 0, {0: [unit_v(1, 0)], 1: [unit_v(1, 1)],
                2: [unit_qk(1, 0, "q")], 3: [unit_qk(1, 0, "k")]})
    attn(1, 0, {0: [unit_qk(2, 0, "q")], 2: [unit_qk(2, 0, "k")]})
    attn(2, 0, {0: [unit_qk(3, 0, "q")], 2: [unit_qk(3, 0, "k")]})
    attn(3, 0, {0: [unit_qk(0, 1, "q")], 2: [unit_qk(0, 1, "k")]})
    attn(0, 1, {0: [unit_v(2, 0)], 1: [unit_v(2, 1)], 2: [unit_v(3, 0)],
                3: [unit_v(3, 1)], 5: [unit_qk(1, 1, "q")],
                7: [unit_qk(1, 1, "k")]})
    attn(1, 1, {2: [unit_qk(2, 1, "q")], 5: [unit_qk(2, 1, "k")]})
    attn(2, 1, {1: [unit_qk(3, 1, "q")], 3: [unit_qk(3, 1, "k")],
                5: [unit_yp(0, 0)], 7: [unit_yp(0, 1)]})
    attn(3, 1, {2: [unit_yp(0, 2)], 5: [unit_yp(0, 3)]})
    # final output projection (si 4..7): two paired PSUM tiles; pairs 0-2
    # accumulate while pair (3,1)'s normalize chain runs, pair 3 lands last
    yps = []
    for g in range(2):
        yp = ps.tile([P, 1024], F32, tag="st", name=f"ypf{g}")
        for sh in range(2):
            sl = g * 2 + sh
            for p in range(3):
                nc.tensor.matmul(
                    yp[:, sh * 512:(sh + 1) * 512],
                    ot_sb[p, 1][:, sl * P:(sl + 1) * P], wo_sb[p][:],
                    start=(p == 0), stop=False, skip_group_check=True)
        yps.append(yp)
    # keep TensorE warm (HAM 8/8) while the last normalize chain runs
    for i in range(10):
        wu = ps.tile([P, 512], F32, tag="yw", bufs=1, name=f"kw{i}")
        nc.tensor.matmul(wu[:], wu_src[:, 0:P], wu_src[:],
                         start=True, stop=True, skip_group_check=True)
    engs = [nc.sync, nc.scalar, nc.sync, nc.scalar]
    for g in range(2):
        for sh in range(2):
            sl = g * 2 + sh
            si = 4 + sl
            nc.tensor.matmul(
                yps[g][:, sh * 512:(sh + 1) * 512],
                ot_sb[3, 1][0:DV, sl * P:(sl + 1) * P], wo_sb[3][0:DV, :],
                start=False, stop=False, skip_group_check=True)
            nc.tensor.matmul(
                yps[g][:, sh * 512:(sh + 1) * 512],
                tmp31[0][:, sl * P:(sl + 1) * P], wo3hi_t[:],
                start=False, stop=True, skip_group_check=True)
            yo = work.tile([P, E], F32, tag="yof", name=f"yof{si}", bufs=2)
            nc.vector.tensor_add(
                yo[:], yps[g][:, sh * 512:(sh + 1) * 512], bob_t[:])
            engs[sl].dma_start(d["y"][si * P:(si + 1) * P, :], yo[:])


def _build():
    nc = bacc.Bacc("TRN2", target_bir_lowering=False, debug=False)
    d = {
        "xt": nc.dram_tensor("xt", [E, S], BF16, kind="ExternalInput").ap(),
        "wq": nc.dram_tensor("wq", [E, HD], BF16, kind="ExternalInput").ap(),
        "wk": nc.dram_tensor("wk", [E, HD], BF16, kind="ExternalInput").ap(),
        "wv": nc.dram_tensor("wv", [E, HD], BF16, kind="ExternalInput").ap(),
        "wo": nc.dram_tensor("wo", [HD, E], BF16, kind="ExternalInput").ap(),
        "tri2": nc.dram_tensor("tri2", [P, 2 * P], BF16, kind="ExternalInput").ap(),
        "bq": nc.dram_tensor("bq", [P, NPAIR], F32, kind="ExternalInput").ap(),
        "bk": nc.dram_tensor("bk", [P, NPAIR], F32, kind="ExternalInput").ap(),
        "bob": nc.dram_tensor("bob", [P, E], F32, kind="ExternalInput").ap(),
        "wo3hi": nc.dram_tensor("wo3hi", [DV, E], BF16, kind="ExternalInput").ap(),
        "y": nc.dram_tensor("y", [S, E], F32, kind="ExternalOutput").ap(),
    }
    with tile.TileContext(nc) as tc:
        with tc.tile_pool(name="const", bufs=1) as const, \
             tc.tile_pool(name="work", bufs=3) as work, \
             tc.tile_pool(name="ps", bufs=2, space="PSUM") as ps:
            _body(nc, tc, const, work, ps, d)
    nc.compile()
    return nc


def get_nc():
    global _COMPILED
    if _COMPILED is None:
        _COMPILED = _build()
    return _COMPILED


def _prep_in_maps(X, Wq, bq, Wk, bk, Wv, bv, Wo, bo):
    f = np.float32
    bf = ml_dtypes.bfloat16
    Wof = np.asarray(Wo, f).reshape(HD, E)
    # A@(V + 1 bv^T)/d = A@V/d + bv exactly (the ones-column denominator
    # divides out), so bv contributes bv_concat @ Wo to every output row.
    bo_eff = np.asarray(bo, f).reshape(E) + np.asarray(bv, f).reshape(HD) @ Wof
    shared = {
        "wq": np.ascontiguousarray(
            np.transpose(np.asarray(Wq, f), (1, 0, 2)).reshape(E, HD).astype(bf)),
        "wk": np.ascontiguousarray(
            np.transpose(np.asarray(Wk, f), (1, 0, 2)).reshape(E, HD).astype(bf)),
        "wv": np.ascontiguousarray(
            np.transpose(np.asarray(Wv, f), (1, 0, 2)).reshape(E, HD).astype(bf)),
        "wo": np.ascontiguousarray(Wof.astype(bf)),
        "bq": np.ascontiguousarray(np.asarray(bq, f).reshape(HD).reshape(NPAIR, P).T),
        "bk": np.ascontiguousarray(np.asarray(bk, f).reshape(HD).reshape(NPAIR, P).T),
        "bob": np.ascontiguousarray(np.broadcast_to(bo_eff.reshape(1, E), (P, E))),
        "wo3hi": np.ascontiguousarray(Wof[HD - DV:HD, :].astype(bf)),
    }
    # 0/1 keep-mask for the diagonal 128x128 triangle (keep k <= q), twice
    # side by side so one DVE op covers both heads
    keep = np.triu(np.ones((P, P), dtype=f))
    shared["tri2"] = np.ascontiguousarray(np.tile(keep, (1, 2)).astype(bf))
    Xf = np.asarray(X, f)
    in_maps = []
    for b in range(B):
        m = dict(shared)
        m["xt"] = np.ascontiguousarray(Xf[b].T.astype(bf))
        in_maps.append(m)
    return in_maps


def kernel(X, Wq, bq, Wk, bk, Wv, bv, Wo, bo):
    nc = get_nc()
    in_maps = _prep_in_maps(X, Wq, bq, Wk, bk, Wv, bv, Wo, bo)
    res = bass_utils.run_bass_kernel_spmd(nc, in_maps, core_ids=list(range(NCORES)))
    return np.stack([res.results[b]["y"] for b in range(B)], axis=0).astype(np.float32)


def run_traced(X, Wq, bq, Wk, bk, Wv, bv, Wo, bo):
    """Like kernel() but with NTFF profiling; returns (out, exec_time_ns)."""
    nc = get_nc()
    in_maps = _prep_in_maps(X, Wq, bq, Wk, bk, Wv, bv, Wo, bo)
    res = bass_utils.run_bass_kernel_spmd(
        nc, in_maps, core_ids=list(range(NCORES)), trace=True)
    out = np.stack([res.results[b]["y"] for b in range(B)], axis=0).astype(np.float32)
    return out, res.exec_time_ns
